# revision 47
# baseline (speedup 1.0000x reference)
"""EnhanceSelfAttention (B=2, S=2048, C=1024, H=16, D=64) on 8 trn2 cores.

Sharding: core c -> batch b = c // 4, head group g = c % 4 (heads 4g..4g+3).
Each core computes its 4 heads end-to-end plus a partial output projection
(rows of w_out for its heads); host sums the 4 partials per batch.

Mixed-precision fp8/fp16 design (all matmul accumulation fp32 in PSUM):
  - QKV projection mostly in fp8e4m3 with DoubleRow (256-row contraction
    slab pairs, 2x PE throughput); weights host-scaled by 32 into the
    e4m3 normal range, rescaled by 1/32 on the PSUM->SBUF move.
  - Short causal rows average few values so fp8 noise doesn't cancel:
    q/k for seq cols 0:512 and the whole chunk-0 (q rows 0:511)
    attention path (V projection, probabilities, attnT, out-projection)
    run in fp16.  Measured absmax/scale ~4.2e-3 vs the 2e-2 gate.
  - RoPE on qT/kT: pair-split layout via host-permuted weight columns,
    32-row block swap via SBUF-SBUF DMA, DVE multiply-adds.
  - scoresT[k_s, q_s] = K Q^T per head in fp16 (row-packed head pairs on
    the PE via tile_position), causal mask on the diagonal 128x128 block
    via identity-matmul of a -1e9 triangle, exp on ACT (scale=1/8 fused).
  - chunks 1-3: probabilities -> fp8 DoubleRow slabs, PV in fp8-DR over
    k-tile pairs (denominator via a 1/16 ones column -> attnT x16 fp8),
    out-projection fp8-DR (w_out x32) -> fp16 y (x 1/512, host sums).

Schedule (built around the ACT-bound softmax, ~64us of exp):
  - Minimal inline prefix: q0/k0 QKV groups (PSUM from the scores pool),
    then chunk-0 scores/exp start immediately; chunk-0's V projection and
    PVs are deferred until after its exps are queued.
  - All remaining PE work (q1/k1 + half-1 QKV group parts, V-projection
    parts, per-chunk output projections) drains through a weighted FIFO,
    one small closure per scores step, riding the per-step ACT slack.
    Hook psums use a dedicated 2-bank pool so the scores/exp PSUM ring
    never couples to hook-op readers; FIFO order keeps every hook-psum
    ring slot's reader emitted before the slot is reused (WAW safety).
  - PV runs 2 tiles behind scores so a pair's first PV never head-blocks
    the PE queue on the previous pair's normalization.
  - PSUM budget (8 banks): scores ring 2x[128,1024] (4) + PV
    accumulators 2x[65,512] (2) + hook pool 2x[128,512] (2).
  - Input DMAs split scalar/sync queues; y DMAs and RoPE swaps on sync;
    ACT exp table preloaded at t=0.
"""

import sys

if "/opt/trn_rl_repo" not in sys.path:
    sys.path.insert(0, "/opt/trn_rl_repo")

import numpy as np

import concourse.bacc as bacc
import concourse.bass as bass
import concourse.tile as tile
from concourse import mybir
from concourse.bass_utils import run_bass_kernel_spmd

B, S, C = 2, 2048, 1024
H, D = 16, 64
TEMP = 1e4
N_CORES = 8
HPC = 4            # heads per core
P = 128
NQC = S // 512     # 4 q-chunks of 512
KT = S // P        # 16 k-tiles
W_SCALE = 32.0     # fp8 range scaling for weights
DEN_SCALE = 16.0   # attnT upscale via 1/16 ones column
OUT_SCALE = W_SCALE * DEN_SCALE  # net scale on po; y copy divides it out

f32 = mybir.dt.float32
bf16 = mybir.dt.bfloat16
fp16 = mybir.dt.float16
fp8 = mybir.dt.float8e4
DR = mybir.MatmulPerfMode.DoubleRow

_NC = None


def _build():
    nc = bacc.Bacc("TRN2", target_bir_lowering=False, debug=False)

    xP = nc.dram_tensor("xP", [4, P, 2, S], fp8, kind="ExternalInput").ap()
    xF = nc.dram_tensor("xF", [P, 8, 512], fp16, kind="ExternalInput").ap()
    wq = nc.dram_tensor("wq", [P, 2, 1024], fp8, kind="ExternalInput").ap()
    wk = nc.dram_tensor("wk", [P, 2, 1024], fp8, kind="ExternalInput").ap()
    wqf = nc.dram_tensor("wqf", [P, 2048], fp16, kind="ExternalInput").ap()
    wkf = nc.dram_tensor("wkf", [P, 2048], fp16, kind="ExternalInput").ap()
    wv = nc.dram_tensor("wv", [P, 2080], fp8, kind="ExternalInput").ap()
    wvf = nc.dram_tensor("wvf", [P, 2080], fp16, kind="ExternalInput").ap()
    wo = nc.dram_tensor("wo", [P, 2, 1024], fp8, kind="ExternalInput").ap()
    wof = nc.dram_tensor("wof", [P, 2, 1024], fp16, kind="ExternalInput").ap()
    qb = nc.dram_tensor("qb", [2, P, 1], f32, kind="ExternalInput").ap()
    vb = nc.dram_tensor("vb", [1, 260], f32, kind="ExternalInput").ap()
    cosT = nc.dram_tensor("cosT", [P, S], fp16, kind="ExternalInput").ap()
    sinT = nc.dram_tensor("sinT", [P, S], fp16, kind="ExternalInput").ap()
    tri = nc.dram_tensor("tri", [P, P], bf16, kind="ExternalInput").ap()
    idn = nc.dram_tensor("idn", [P, P], bf16, kind="ExternalInput").ap()
    y = nc.dram_tensor("y", [S, C], fp16, kind="ExternalOutput").ap()

    with tile.TileContext(nc) as tc:
        _body(nc, tc, xP, xF, wq, wk, wqf, wkf, wv, wvf, wo, wof,
              qb, vb, cosT, sinT, tri, idn, y)
    nc.compile()
    return nc


def _body(nc, tc, xP, xF, wq, wk, wqf, wkf, wv, wvf, wo, wof,
          qb, vb, cosT, sinT, tri, idn, y):
    from contextlib import ExitStack

    with ExitStack() as ctx:
        consts = ctx.enter_context(tc.tile_pool(name="consts", bufs=1))

        x_sb = [consts.tile([P, 2, S], fp8, tag=f"x{cb}", name=f"x{cb}")
                for cb in range(4)]
        xf_sb = consts.tile([P, 8, 512], fp16, tag="xf", name="xf")
        wq_sb = consts.tile([P, 2, 1024], fp8, tag="wq", name="wq")
        wk_sb = consts.tile([P, 2, 1024], fp8, tag="wk", name="wk")
        wqf_sb = consts.tile([P, 2048], fp16, tag="wqf", name="wqf")
        wkf_sb = consts.tile([P, 2048], fp16, tag="wkf", name="wkf")
        wv_sb = consts.tile([P, 2080], fp8, tag="wv", name="wv")
        wvf_sb = consts.tile([P, 2080], fp16, tag="wvf", name="wvf")
        wo_sb = consts.tile([P, 2, 1024], fp8, tag="wo", name="wo")
        wof_sb = consts.tile([P, 2, 1024], fp16, tag="wof", name="wof")
        cos_sb = consts.tile([P, S], fp16, tag="cos", name="cos")
        sin_sb = consts.tile([P, S], fp16, tag="sin", name="sin")
        tri_sb = consts.tile([P, P], bf16, tag="tri", name="tri")
        idn_sb = consts.tile([P, P], bf16, tag="idn", name="idn")
        qb_sb = [consts.tile([P, 1], f32, tag=f"qb{t}", name=f"qb{t}") for t in range(2)]
        vb_sb = consts.tile([P, 260], f32, tag="vb", name="vb")

        qrot = [consts.tile([P, S], fp16, tag=f"qrot{t}", name=f"qrot{t}") for t in range(2)]
        krot = [consts.tile([P, S], fp16, tag=f"krot{t}", name=f"krot{t}") for t in range(2)]
        # v packed for DoubleRow PV (chunks 1-3): tile per k-tile pair,
        # slabs = k parity, head h at cols [80h : 80h+65) (80h+64 = 1/16 col)
        v_pk = [consts.tile([P, 2, 320], fp8, tag=f"v{sp}", name=f"v{sp}")
                for sp in range(KT // 2)]
        # fp16 v for chunk 0 (k-tiles 0..3), 65-col-per-head layout
        v16 = [consts.tile([P, 260], fp16, tag=f"v16_{st}", name=f"v16_{st}")
               for st in range(4)]

        # ---- input DMAs, batch A (needed for half-0 QKV + chunks 0-1)
        nc.scalar.dma_start(wq_sb[:], wq[:])
        nc.scalar.dma_start(wk_sb[:], wk[:])
        for t in range(2):
            nc.scalar.dma_start(qb_sb[t][:], qb[t])
        nc.scalar.dma_start(
            vb_sb[:],
            bass.AP(tensor=vb.tensor, offset=vb.offset, ap=[[0, P], [1, 260]]),
        )
        nc.sync.dma_start(wqf_sb[:], wqf[:])
        nc.sync.dma_start(xf_sb[:, 0:4, :], xF[:, 0:4, :])
        nc.sync.dma_start(xf_sb[:, 4:8, :], xF[:, 4:8, :])
        nc.sync.dma_start(wkf_sb[:], wkf[:])
        for cb in range(4):
            nc.sync.dma_start(x_sb[cb][:, :, 512:1024], xP[cb][:, :, 512:1024])
        nc.scalar.dma_start(wvf_sb[:], wvf[:])
        nc.scalar.dma_start(wv_sb[:], wv[:])
        nc.sync.dma_start(cos_sb[:, 0:1024], cosT[:, 0:1024])
        nc.sync.dma_start(sin_sb[:, 0:1024], sinT[:, 0:1024])
        nc.scalar.dma_start(tri_sb[:], tri[:])
        nc.scalar.dma_start(idn_sb[:], idn[:])

        def dma_batch_b():
            for cb in range(4):
                nc.sync.dma_start(x_sb[cb][:, :, 1024:2048], xP[cb][:, :, 1024:2048])
            nc.sync.dma_start(cos_sb[:, 1024:2048], cosT[:, 1024:2048])
            nc.sync.dma_start(sin_sb[:, 1024:2048], sinT[:, 1024:2048])
            nc.sync.dma_start(wo_sb[:], wo[:])
            nc.sync.dma_start(wof_sb[:], wof[:])

        scp = ctx.enter_context(tc.tile_pool(name="scps", bufs=2, space="PSUM"))
        pvp = ctx.enter_context(tc.tile_pool(name="pvps", bufs=1, space="PSUM"))
        pop = ctx.enter_context(tc.tile_pool(name="pops", bufs=1, space="PSUM"))
        tmp = ctx.enter_context(tc.tile_pool(name="qktmp", bufs=3))
        swp = ctx.enter_context(tc.tile_pool(name="qkswp", bufs=3))
        exp_pool = ctx.enter_context(tc.tile_pool(name="expool", bufs=4))
        ex16_pool = ctx.enter_context(tc.tile_pool(name="ex16pool", bufs=4))
        atp = ctx.enter_context(tc.tile_pool(name="atp", bufs=2))
        rcp_pool = ctx.enter_context(tc.tile_pool(name="rcppool", bufs=3))
        rbp = ctx.enter_context(tc.tile_pool(name="rbp", bufs=3))
        yp = ctx.enter_context(tc.tile_pool(name="ybuf", bufs=2))

        # ---------------- QKV + RoPE producers -----------------
        def qk_group(kind, t, half):
            if kind == "q":
                wsb, wfsb, rot = wq_sb, wqf_sb, qrot
            else:
                wsb, wfsb, rot = wk_sb, wkf_sb, krot
            s0 = half * 1024
            ps = scp.tile([P, 1024], f32, tag="sc", name="sc")
            for n2 in range(2):
                lo = s0 + n2 * 512
                if lo == 0:
                    # fp16 path: seq cols 0:512 (short causal rows downstream)
                    for k8 in range(8):
                        nc.tensor.matmul(
                            ps[:, 0:512],
                            wfsb[:, k8 * 256 + t * P: k8 * 256 + (t + 1) * P],
                            xf_sb[:, k8, :],
                            start=(k8 == 0), stop=(k8 == 7),
                        )
                else:
                    for cb in range(4):
                        nc.tensor.matmul(
                            ps[:, n2 * 512:(n2 + 1) * 512],
                            wsb[:, :, cb * 256 + t * P: cb * 256 + (t + 1) * P],
                            x_sb[cb][:, :, lo:lo + 512],
                            start=(cb == 0), stop=(cb == 3),
                            perf_mode=DR,
                        )
            span = slice(s0, s0 + 1024)
            base = tmp.tile([P, 1024], fp16, tag="base", name="base")
            if kind == "q":
                nc.vector.tensor_scalar(
                    base[:], ps[:], 1.0 / W_SCALE, qb_sb[t][:, 0:1],
                    mybir.AluOpType.mult, mybir.AluOpType.add,
                )
            else:
                nc.vector.tensor_scalar_mul(base[:], ps[:], 1.0 / W_SCALE)
            sw = swp.tile([P, 1024], fp16, tag="sw", name="sw")
            for blk in range(4):
                sb_, db = 32 * (blk ^ 1), 32 * blk
                nc.sync.dma_start(sw[db:db + 32, :], base[sb_:sb_ + 32, :])
            nc.vector.tensor_mul(base[:], base[:], cos_sb[:, span])
            nc.vector.tensor_mul(sw[:], sw[:], sin_sb[:, span])
            nc.vector.tensor_add(rot[t][:, span], base[:], sw[:])

        def v_group(st):
            ps = scp.tile([P, 1024], f32, tag="sc", name="sc")
            if st < 4:
                # fp16 path feeding chunk 0
                for k8 in range(8):
                    nc.tensor.matmul(
                        ps[:, 0:260],
                        xf_sb[:, k8, st * P:(st + 1) * P],
                        wvf_sb[:, k8 * 260:(k8 + 1) * 260],
                        start=(k8 == 0), stop=(k8 == 7),
                    )
                nc.vector.scalar_tensor_tensor(
                    v16[st][:], ps[:, 0:260], 1.0 / W_SCALE, vb_sb[:],
                    mybir.AluOpType.mult, mybir.AluOpType.add,
                )
            else:
                for sl in range(8):
                    nc.tensor.matmul(
                        ps[:, 0:260],
                        x_sb[sl // 2][:, sl % 2, st * P:(st + 1) * P],
                        wv_sb[:, sl * 260:(sl + 1) * 260],
                        start=(sl == 0), stop=(sl == 7),
                    )
            dst = v_pk[st // 2][:, st % 2, :].rearrange(
                "p (h c) -> p h c", h=4)[:, :, 0:65]
            src = (v16[st][:] if st < 4 else ps[:, 0:260]).rearrange(
                "p (h c) -> p h c", h=4)
            if st < 4:
                nc.vector.tensor_copy(dst, src)
            else:
                vbr = vb_sb[:].rearrange("p (h c) -> p h c", h=4)
                nc.vector.scalar_tensor_tensor(
                    dst, src, 1.0 / W_SCALE, vbr,
                    mybir.AluOpType.mult, mybir.AluOpType.add,
                )

        def qkv_half(half, kinds=("q0", "k0", "q1", "k1"), st0=None):
            sts = iter(range(st0 if st0 is not None else 8 * half, 8 * half + 8))
            for kt_ in kinds:
                qk_group(kt_[0], int(kt_[1]), half)
                v_group(next(sts))
                v_group(next(sts))

        def group_closures(kind, t, half, sts):
            """Closures emitting one half-1 qk group + its two V groups."""
            return [lambda: qk_group(kind, t, half),
                    lambda: v_group(sts[0]),
                    lambda: v_group(sts[1])]

        def drain_one(fifo):
            """Pop hook closures until one unit of PE work has been emitted
            (weight-0 closures carry no PE work and flow through free)."""
            while fifo:
                w, fn = fifo.pop(0)
                fn()
                if w:
                    return

        def drain_until(fifo, label):
            while fifo:
                w, fn = fifo.pop(0)
                fn()
                if fn.__name__ == label or getattr(fn, "label", None) == label:
                    return

        # ---------------- attention (one head-pair of one q-chunk) ------
        def attn_pair(n, pair, at_t, hook=None, tail_hook=None, defer_pv=None):
            """Emit scores/exp/PV/norm for (q-chunk n, head pair); `hook` is
            a FIFO of closures (projections, half-1 QKV group parts, V
            parts) drained one per scores step -- their PE work rides the
            per-step ACT slack instead of creating softmax droughts.
            `tail_hook` fires after the k loop, before the normalization."""
            if hook is None:
                hook = []
            f16 = (n == 0)
            if True:
                nkt = 4 * n + 4
                pv_ps = [pvp.tile([65, 512], f32, tag=f"pv{hh}", name=f"pv{hh}")
                         for hh in range(2)]

                def scores_step(kt, ex_t):
                    diag = (kt // 4 == n)
                    off = P * (kt % 4) if diag else 0
                    sc = scp.tile([P, 1024], f32, tag="sc", name="sc")
                    for hh in range(2):
                        nc.tensor.matmul(
                            sc[:, hh * 512 + off:(hh + 1) * 512],
                            krot[pair][64 * hh:64 * hh + 64, kt * P:(kt + 1) * P],
                            qrot[pair][64 * hh:64 * hh + 64,
                                       n * 512 + off:(n + 1) * 512],
                            start=True, stop=not diag,
                            tile_position=(64 * hh, 0),
                        )
                    if diag:
                        for hh in range(2):
                            nc.tensor.matmul(
                                sc[:, hh * 512 + off:hh * 512 + off + P],
                                idn_sb[:], tri_sb[:],
                                start=False, stop=True,
                            )
                    ex_ap = ex_t[:] if f16 else ex_t[:, kt % 2, :]
                    if diag:
                        exr = ex_ap.rearrange("p (h c) -> p h c", h=2)[:, :, off:]
                        scr = sc[:].rearrange("p (h c) -> p h c", h=2)[:, :, off:]
                        nc.scalar.activation(
                            exr, scr, mybir.ActivationFunctionType.Exp,
                            scale=0.125,
                        )
                        if not f16 and kt % 2 == 1:
                            moff = P * 2 * ((kt % 4) // 2)
                            gap = ex_t[:, 1, :].rearrange(
                                "p (h c) -> p h c", h=2)[:, :, moff:moff + P]
                            nc.gpsimd.memset(gap, 0)
                    else:
                        nc.scalar.activation(
                            ex_ap, sc[:], mybir.ActivationFunctionType.Exp,
                            scale=0.125,
                        )

                if f16:
                    # per-k-tile fp16 PV, 1-tile software pipeline
                    def pv16_step(kt, ex_t):
                        off = P * (kt % 4)
                        for hh in range(2):
                            h = 2 * pair + hh
                            nc.tensor.matmul(
                                pv_ps[hh][:, off:512],
                                v16[kt][:, 65 * h:65 * h + 65],
                                ex_t[:, hh * 512 + off:(hh + 1) * 512],
                                start=(kt == 0), stop=(kt == nkt - 1),
                            )

                    if defer_pv is not None:
                        # scores/exp first; V projection + all PVs after, so
                        # nothing sits between the prefix and the first exp
                        exs = []
                        for kt in range(nkt):
                            ex_t = ex16_pool.tile([P, 1024], fp16,
                                                  tag="ex16", name="ex16")
                            scores_step(kt, ex_t)
                            exs.append((kt, ex_t))
                        for fn in defer_pv:
                            fn()
                        for kt, ex_t in exs:
                            pv16_step(kt, ex_t)
                    else:
                        pend = []
                        for kt in range(nkt):
                            ex_t = ex16_pool.tile([P, 1024], fp16,
                                                  tag="ex16", name="ex16")
                            scores_step(kt, ex_t)
                            if kt >= 1:
                                drain_one(hook)
                            pend.append((kt, ex_t))
                            if len(pend) > 2:
                                pv16_step(*pend.pop(0))
                        for pp_ in pend:
                            pv16_step(*pp_)
                else:
                    def pv_step(sp, ex_t):
                        diag = (sp // 2 == n)
                        moff = P * 2 * (sp % 2) if diag else 0
                        for hh in range(2):
                            h = 2 * pair + hh
                            nc.tensor.matmul(
                                pv_ps[hh][:, moff:512],
                                v_pk[sp][:, :, 80 * h:80 * h + 65],
                                ex_t[:, :, hh * 512 + moff:(hh + 1) * 512],
                                start=(sp == 0), stop=(sp == 2 * n + 1),
                                perf_mode=DR,
                            )

                    pend = []
                    ex_t = None
                    for kt in range(nkt):
                        if kt % 2 == 0:
                            ex_t = exp_pool.tile([P, 2, 1024], fp8, tag="ex", name="ex")
                        scores_step(kt, ex_t)
                        if kt >= 1:
                            drain_one(hook)
                        if kt % 2 == 1:
                            pend.append((kt // 2, ex_t))
                            if len(pend) > 2:
                                pv_step(*pend.pop(0))
                    for pp_ in pend:
                        pv_step(*pp_)

                if tail_hook is not None:
                    tail_hook()
                for hh in range(2):
                    dc = rcp_pool.tile([1, 512], f32, tag="dc", name="dc")
                    nc.vector.tensor_copy(dc[:], pv_ps[hh][64:65, :])
                    rc = rcp_pool.tile([1, 512], f32, tag="rc", name="rc")
                    nc.vector.reciprocal_approx_fast(rc[:], dc[:])
                    rb = rbp.tile([64, 512], f32, tag="rb", name="rb")
                    nc.gpsimd.partition_broadcast(rb[:], rc[0:1, :])
                    nc.vector.tensor_mul(
                        at_t[64 * hh:64 * hh + 64, pair, :],
                        pv_ps[hh][0:64, :],
                        rb[:],
                    )

        def make_proj(n, at_t, last=False):
            f16 = (n == 0)

            def proj_i(i):
                po = pop.tile([P, C], f32, tag="po", name="po")
                for c2 in range(2):
                    if f16:
                        for kk in range(2):
                            nc.tensor.matmul(
                                po[:, c2 * 512:(c2 + 1) * 512],
                                at_t[:, kk, i * P:(i + 1) * P],
                                wof_sb[:, kk, c2 * 512:(c2 + 1) * 512],
                                start=(kk == 0), stop=(kk == 1),
                            )
                    else:
                        nc.tensor.matmul(
                            po[:, c2 * 512:(c2 + 1) * 512],
                            at_t[:, :, i * P:(i + 1) * P],
                            wo_sb[:, :, c2 * 512:(c2 + 1) * 512],
                            start=True, stop=True,
                            perf_mode=DR,
                        )
                yo = yp.tile([P, C], fp16, tag="yo", name="yo")
                if last and i % 2 == 0:
                    nc.scalar.activation(
                        yo[:], po[:], mybir.ActivationFunctionType.Copy,
                        scale=1.0 / OUT_SCALE,
                    )
                else:
                    nc.vector.tensor_scalar_mul(yo[:], po[:], 1.0 / OUT_SCALE)
                r0 = n * 512 + i * P
                nc.sync.dma_start(y[r0:r0 + P, :], yo[:])

            return [lambda i=i: proj_i(i) for i in range(4)]

        def new_at(n):
            return atp.tile([P, 2, 512], fp16 if n == 0 else fp8,
                            tag="at", name="at")

        # ---------------- schedule -----------------------------
        # Minimal inline prefix (q0, k0 + V16) before chunk-0 attention;
        # everything else (q1/k1/half-1 group parts, V8 parts, chunk
        # projections) drains through a global weighted FIFO, one PE
        # closure per scores step, riding the ACT-bound softmax slack.
        # FIFO order keeps every hook-psum ring slot's reader emitted
        # before the slot is reused (WAW safety at hps bufs=2).
        q0mm, q0rope = qk_parts("q", 0, 0, nc.scalar, prefix=True)
        k0mm, k0rope = qk_parts("k", 0, 0, nc.sync, prefix=True)
        q1mm, q1rope = qk_parts("q", 1, 0, nc.scalar)
        k1mm, k1rope = qk_parts("k", 1, 0, nc.sync)
        hq0mm, hq0rope = qk_parts("q", 0, 1, nc.sync)
        hk0mm, hk0rope = qk_parts("k", 0, 1, nc.sync)
        hq1mm, hq1rope = qk_parts("q", 1, 1, nc.sync)
        hk1mm, hk1rope = qk_parts("k", 1, 1, nc.sync)
        v8 = {st: v_parts(st) for st in range(4, 16)}

        q0mm(0); q0mm(1)
        q0rope(0, 512); q0rope(512, 1024)
        k0mm(0); k0mm(1)
        k0rope(0, 512); k0rope(512, 1024)
        dma_batch_b()

        def lab(fn, name):
            fn.label = name
            return fn

        fifo = []
        fifo += [(1, v8[4][0]), (1, v8[4][1]), (1, v8[5][0]), (1, v8[5][1])]
        fifo += [(1, v8[6][0]), (1, v8[6][1]), (1, v8[7][0]), (1, v8[7][1])]
        at0 = new_at(0)

        def q1k1():
            q1mm(0); q1mm(1)
            q1rope(0, 512); q1rope(512, 1024)
            k1mm(0); k1mm(1)
            k1rope(0, 512); k1rope(512, 1024)

        attn_pair(0, 0, at0, hook=fifo,
                  defer_pv=[q1k1] + [lambda st=st: v_group(st) for st in range(4)])
        attn_pair(0, 1, at0, hook=fifo)

        at1 = new_at(1)
        proj0 = make_proj(0, at0)
        fifo += [(1, lambda: hq0mm(0)), (1, lambda: hq0mm(1)),
                 (0, lambda: hq0rope(0, 512)), (0, lambda: hq0rope(512, 1024)),
                 (1, lambda: hk0mm(0)), (1, lambda: hk0mm(1)),
                 (0, lambda: hk0rope(0, 512)),
                 (0, lab(lambda: hk0rope(512, 1024), "hk0done"))]
        fifo += [(1, p) for p in proj0]
        fifo += [(1, v8[8][0])]
        attn_pair(1, 0, at1, hook=fifo)
        attn_pair(1, 1, at1, hook=fifo)
        drain_until(fifo, "hk0done")

        at2 = new_at(2)
        proj1 = make_proj(1, at1)
        fifo += [(1, lambda: hq1mm(0)), (1, lambda: hq1mm(1)),
                 (0, lambda: hq1rope(0, 512)), (0, lambda: hq1rope(512, 1024)),
                 (1, lambda: hk1mm(0)), (1, lambda: hk1mm(1)),
                 (0, lambda: hk1rope(0, 512)),
                 (0, lab(lambda: hk1rope(512, 1024), "hk1done"))]
        fifo += [(1, v8[8][1]), (1, v8[9][0]), (1, v8[9][1]),
                 (1, v8[10][0]), (1, v8[10][1]), (1, v8[11][0]), (1, v8[11][1])]
        attn_pair(2, 0, at2, hook=fifo)
        drain_until(fifo, "hk1done")
        fifo += [(1, p) for p in proj1]
        fifo += [(1, v8[12][0]), (1, v8[12][1]), (1, v8[13][0]), (1, v8[13][1]),
                 (1, v8[14][0]), (1, v8[14][1]), (1, v8[15][0]), (1, v8[15][1])]
        attn_pair(2, 1, at2, hook=fifo)

        at3 = new_at(3)
        proj2 = make_proj(2, at2)
        fifo += [(1, p) for p in proj2]
        attn_pair(3, 0, at3, hook=fifo)
        attn_pair(3, 1, at3, hook=fifo)
        while fifo:
            drain_one(fifo)
        for p in make_proj(3, at3, last=True):
            p()


def _host_inputs(x, w_qkv, q_bias, v_bias, w_out):
    """Build the 8 per-core input maps."""
    import ml_dtypes
    e4m3 = ml_dtypes.float8_e4m3fn
    half = D // 2
    perm64 = np.empty(D, dtype=np.int64)
    perm64[:half] = 2 * np.arange(half)
    perm64[half:] = 2 * np.arange(half) + 1

    dim_t = (TEMP ** (np.arange(half, dtype=np.float32) / half)).astype(np.float32)
    ang = (np.arange(S, dtype=np.float32)[None, :] / dim_t[:, None]).astype(np.float32)
    cos32 = np.cos(ang).astype(np.float32)
    sin32 = np.sin(ang).astype(np.float32)
    cosT = np.tile(cos32, (4, 1)).astype(np.float16)
    sinT = np.concatenate([-sin32, sin32, -sin32, sin32], axis=0).astype(np.float16)

    r = np.arange(P)
    tri = np.where(r[None, :] >= r[:, None], 0.0, -1e9).astype(ml_dtypes.bfloat16)
    idn = np.eye(P, dtype=np.float32).astype(ml_dtypes.bfloat16)

    Wq = w_qkv[:, 0:C]
    Wk = w_qkv[:, C:2 * C]
    Wv = w_qkv[:, 2 * C:3 * C]

    def pack_dr(A):
        # [1024, 256] -> [128, 2, 1024]: out[i, j, cb*256 + m] = A[256cb+i+128j, m]
        return np.ascontiguousarray(
            A.reshape(4, 2, P, 256).transpose(2, 1, 0, 3).reshape(P, 2, 1024)
        )

    def pack16(A):
        # [1024, 256] -> [128, 2048]: out[i, k*256 + m] = A[128k+i, m]
        return np.ascontiguousarray(
            A.reshape(8, P, 256).transpose(1, 0, 2).reshape(P, 2048)
        )

    in_maps = []
    for core in range(N_CORES):
        b, g = core // HPC, core % HPC
        h0 = HPC * g
        cols = np.concatenate([64 * h + perm64 for h in range(h0, h0 + HPC)])
        vcols = np.arange(64 * h0, 64 * h0 + 256)

        xb = np.ascontiguousarray(x[b].T)                     # [C, S]
        xP = np.ascontiguousarray(
            xb.reshape(4, 2, P, S).transpose(0, 2, 1, 3)
        ).astype(e4m3)                                        # [4, 128, 2, S]
        xF = np.ascontiguousarray(
            xb[:, 0:512].reshape(8, P, 512).transpose(1, 0, 2)).astype(np.float16)

        Aq = W_SCALE * Wq[:, cols]
        Ak = W_SCALE * Wk[:, cols]
        wq_pk = pack_dr(Aq).astype(e4m3)
        wk_pk = pack_dr(Ak).astype(e4m3)
        wqf_pk = pack16(Aq).astype(np.float16)
        wkf_pk = pack16(Ak).astype(np.float16)

        wv260 = np.zeros((C, 260), dtype=np.float32)
        vb260 = np.zeros((1, 260), dtype=np.float32)
        wvc = Wv[:, vcols]
        vbc = v_bias[vcols]
        for hh_ in range(4):
            wv260[:, 65 * hh_:65 * hh_ + 64] = wvc[:, 64 * hh_:64 * hh_ + 64]
            vb260[0, 65 * hh_:65 * hh_ + 64] = vbc[64 * hh_:64 * hh_ + 64]
            vb260[0, 65 * hh_ + 64] = 1.0 / DEN_SCALE
        Av = W_SCALE * wv260
        # [1024, 260] -> [128, 2080]: out[i, (2cb+j)*260+m] = Av[256cb+i+128j, m]
        wv_pk = np.ascontiguousarray(
            Av.reshape(4, 2, P, 260).transpose(2, 0, 1, 3).reshape(P, 2080)
        ).astype(e4m3)
        # fp16: out[i, k*260+m] = Av[128k+i, m]
        wvf_pk = np.ascontiguousarray(
            Av.reshape(8, P, 260).transpose(1, 0, 2).reshape(P, 2080)
        ).astype(np.float16)

        Ao = (W_SCALE * w_out[64 * h0:64 * h0 + 256, :]).reshape(2, P, C).transpose(1, 0, 2)
        wo_pk = np.ascontiguousarray(Ao).astype(e4m3)          # [128, 2, 1024]
        wof_pk = np.ascontiguousarray(Ao).astype(np.float16)

        in_maps.append({
            "xP": xP, "xF": xF,
            "wq": wq_pk, "wk": wk_pk, "wqf": wqf_pk, "wkf": wkf_pk,
            "wv": wv_pk, "wvf": wvf_pk, "wo": wo_pk, "wof": wof_pk,
            "qb": np.ascontiguousarray(q_bias[cols].reshape(2, P, 1)),
            "vb": vb260,
            "cosT": cosT, "sinT": sinT, "tri": tri, "idn": idn,
        })
    return in_maps


def kernel(x, w_qkv, q_bias, v_bias, w_out, _trace=False):
    global _NC
    if _NC is None:
        _NC = _build()
    in_maps = _host_inputs(
        np.asarray(x, np.float32), np.asarray(w_qkv, np.float32),
        np.asarray(q_bias, np.float32), np.asarray(v_bias, np.float32),
        np.asarray(w_out, np.float32),
    )
    res = run_bass_kernel_spmd(_NC, in_maps, list(range(N_CORES)), trace=_trace)
    out = np.empty((B, S, C), dtype=np.float32)
    for b in range(B):
        acc = res.results[HPC * b]["y"].astype(np.float32)
        for g in range(1, HPC):
            acc = acc + res.results[HPC * b + g]["y"].astype(np.float32)
        out[b] = acc
    if _trace:
        kernel.last_exec_time_ns = res.exec_time_ns
    return out


# revision 48
# speedup vs baseline: 1.0191x; 1.0191x over previous
"""EnhanceSelfAttention (B=2, S=2048, C=1024, H=16, D=64) on 8 trn2 cores.

Sharding: core c -> batch b = c // 4, head group g = c % 4 (heads 4g..4g+3).
Each core computes its 4 heads end-to-end plus a partial output projection
(rows of w_out for its heads); host sums the 4 partials per batch.

Mixed-precision fp8/fp16 design (all matmul accumulation fp32 in PSUM):
  - QKV projection mostly in fp8e4m3 with DoubleRow (256-row contraction
    slab pairs, 2x PE throughput); weights host-scaled by 32 into the
    e4m3 normal range, rescaled by 1/32 on the PSUM->SBUF move.
  - Short causal rows average few values so fp8 noise doesn't cancel:
    q/k for seq cols 0:512 and the whole chunk-0 (q rows 0:511)
    attention path (V projection, probabilities, attnT, out-projection)
    run in fp16.  Measured absmax/scale ~4.2e-3 vs the 2e-2 gate.
  - RoPE on qT/kT: pair-split layout via host-permuted weight columns,
    32-row block swap via SBUF-SBUF DMA, DVE multiply-adds.
  - scoresT[k_s, q_s] = K Q^T per head in fp16 (row-packed head pairs on
    the PE via tile_position), causal mask on the diagonal 128x128 block
    via identity-matmul of a -1e9 triangle, exp on ACT (scale=1/8 fused).
  - chunks 1-3: probabilities -> fp8 DoubleRow slabs, PV in fp8-DR over
    k-tile pairs (denominator via a 1/16 ones column -> attnT x16 fp8),
    out-projection fp8-DR (w_out x32) -> fp16 y (x 1/512, host sums).

Schedule (built around the ACT-bound softmax, ~64us of exp):
  - Minimal inline prefix: q0/k0 QKV groups (PSUM from the scores pool),
    then chunk-0 scores/exp start immediately; chunk-0's V projection and
    PVs are deferred until after its exps are queued.
  - All remaining PE work (q1/k1 + half-1 QKV group parts, V-projection
    parts, per-chunk output projections) drains through a weighted FIFO,
    one small closure per scores step, riding the per-step ACT slack.
    Hook psums use a dedicated 2-bank pool so the scores/exp PSUM ring
    never couples to hook-op readers; FIFO order keeps every hook-psum
    ring slot's reader emitted before the slot is reused (WAW safety).
  - PV runs 2 tiles behind scores so a pair's first PV never head-blocks
    the PE queue on the previous pair's normalization.
  - PSUM budget (8 banks): scores ring 2x[128,1024] (4) + PV
    accumulators 2x[65,512] (2) + hook pool 2x[128,512] (2).
  - Input DMAs split scalar/sync queues; y DMAs and RoPE swaps on sync;
    ACT exp table preloaded at t=0.
"""

import sys

if "/opt/trn_rl_repo" not in sys.path:
    sys.path.insert(0, "/opt/trn_rl_repo")

import numpy as np

import concourse.bacc as bacc
import concourse.bass as bass
import concourse.tile as tile
from concourse import mybir
from concourse.bass_utils import run_bass_kernel_spmd

B, S, C = 2, 2048, 1024
H, D = 16, 64
TEMP = 1e4
N_CORES = 8
HPC = 4            # heads per core
P = 128
NQC = S // 512     # 4 q-chunks of 512
KT = S // P        # 16 k-tiles
W_SCALE = 32.0     # fp8 range scaling for weights
DEN_SCALE = 16.0   # attnT upscale via 1/16 ones column
OUT_SCALE = W_SCALE * DEN_SCALE  # net scale on po; y copy divides it out

f32 = mybir.dt.float32
bf16 = mybir.dt.bfloat16
fp16 = mybir.dt.float16
fp8 = mybir.dt.float8e4
DR = mybir.MatmulPerfMode.DoubleRow

_NC = None


def _build():
    nc = bacc.Bacc("TRN2", target_bir_lowering=False, debug=False)

    xP = nc.dram_tensor("xP", [4, P, 2, S], fp8, kind="ExternalInput").ap()
    xF = nc.dram_tensor("xF", [P, 8, 512], fp16, kind="ExternalInput").ap()
    wq = nc.dram_tensor("wq", [P, 2, 1024], fp8, kind="ExternalInput").ap()
    wk = nc.dram_tensor("wk", [P, 2, 1024], fp8, kind="ExternalInput").ap()
    wqf = nc.dram_tensor("wqf", [P, 2048], fp16, kind="ExternalInput").ap()
    wkf = nc.dram_tensor("wkf", [P, 2048], fp16, kind="ExternalInput").ap()
    wv = nc.dram_tensor("wv", [P, 2080], fp8, kind="ExternalInput").ap()
    wvf = nc.dram_tensor("wvf", [P, 2080], fp16, kind="ExternalInput").ap()
    wo = nc.dram_tensor("wo", [P, 2, 1024], fp8, kind="ExternalInput").ap()
    wof = nc.dram_tensor("wof", [P, 2, 1024], fp16, kind="ExternalInput").ap()
    qb = nc.dram_tensor("qb", [2, P, 1], f32, kind="ExternalInput").ap()
    vb = nc.dram_tensor("vb", [1, 260], f32, kind="ExternalInput").ap()
    cosT = nc.dram_tensor("cosT", [P, S], fp16, kind="ExternalInput").ap()
    sinT = nc.dram_tensor("sinT", [P, S], fp16, kind="ExternalInput").ap()
    tri = nc.dram_tensor("tri", [P, P], bf16, kind="ExternalInput").ap()
    idn = nc.dram_tensor("idn", [P, P], bf16, kind="ExternalInput").ap()
    y = nc.dram_tensor("y", [S, C], fp16, kind="ExternalOutput").ap()

    with tile.TileContext(nc) as tc:
        _body(nc, tc, xP, xF, wq, wk, wqf, wkf, wv, wvf, wo, wof,
              qb, vb, cosT, sinT, tri, idn, y)
    nc.compile()
    return nc


def _body(nc, tc, xP, xF, wq, wk, wqf, wkf, wv, wvf, wo, wof,
          qb, vb, cosT, sinT, tri, idn, y):
    from contextlib import ExitStack

    with ExitStack() as ctx:
        consts = ctx.enter_context(tc.tile_pool(name="consts", bufs=1))

        x_sb = [consts.tile([P, 2, S], fp8, tag=f"x{cb}", name=f"x{cb}")
                for cb in range(4)]
        xf_sb = consts.tile([P, 8, 512], fp16, tag="xf", name="xf")
        wq_sb = consts.tile([P, 2, 1024], fp8, tag="wq", name="wq")
        wk_sb = consts.tile([P, 2, 1024], fp8, tag="wk", name="wk")
        wqf_sb = consts.tile([P, 2048], fp16, tag="wqf", name="wqf")
        wkf_sb = consts.tile([P, 2048], fp16, tag="wkf", name="wkf")
        wv_sb = consts.tile([P, 2080], fp8, tag="wv", name="wv")
        wvf_sb = consts.tile([P, 2080], fp16, tag="wvf", name="wvf")
        wo_sb = consts.tile([P, 2, 1024], fp8, tag="wo", name="wo")
        wof_sb = consts.tile([P, 2, 1024], fp16, tag="wof", name="wof")
        cos_sb = consts.tile([P, S], fp16, tag="cos", name="cos")
        sin_sb = consts.tile([P, S], fp16, tag="sin", name="sin")
        tri_sb = consts.tile([P, P], bf16, tag="tri", name="tri")
        idn_sb = consts.tile([P, P], bf16, tag="idn", name="idn")
        qb_sb = [consts.tile([P, 1], f32, tag=f"qb{t}", name=f"qb{t}") for t in range(2)]
        vb_sb = consts.tile([P, 260], f32, tag="vb", name="vb")

        qrot = [consts.tile([P, S], fp16, tag=f"qrot{t}", name=f"qrot{t}") for t in range(2)]
        krot = [consts.tile([P, S], fp16, tag=f"krot{t}", name=f"krot{t}") for t in range(2)]
        # v packed for DoubleRow PV (chunks 1-3): tile per k-tile pair,
        # slabs = k parity, head h at cols [80h : 80h+65) (80h+64 = 1/16 col)
        v_pk = [consts.tile([P, 2, 320], fp8, tag=f"v{sp}", name=f"v{sp}")
                for sp in range(KT // 2)]
        # fp16 v for chunk 0 (k-tiles 0..3), 65-col-per-head layout
        v16 = [consts.tile([P, 260], fp16, tag=f"v16_{st}", name=f"v16_{st}")
               for st in range(4)]

        # ---- input DMAs, batch A (needed for half-0 QKV + chunks 0-1)
        nc.scalar.dma_start(wq_sb[:], wq[:])
        nc.scalar.dma_start(wk_sb[:], wk[:])
        for t in range(2):
            nc.scalar.dma_start(qb_sb[t][:], qb[t])
        nc.scalar.dma_start(
            vb_sb[:],
            bass.AP(tensor=vb.tensor, offset=vb.offset, ap=[[0, P], [1, 260]]),
        )
        nc.sync.dma_start(wqf_sb[:], wqf[:])
        nc.sync.dma_start(xf_sb[:, 0:4, :], xF[:, 0:4, :])
        nc.sync.dma_start(xf_sb[:, 4:8, :], xF[:, 4:8, :])
        nc.sync.dma_start(wkf_sb[:], wkf[:])
        for cb in range(4):
            nc.sync.dma_start(x_sb[cb][:, :, 512:1024], xP[cb][:, :, 512:1024])
        nc.scalar.dma_start(wvf_sb[:], wvf[:])
        nc.scalar.dma_start(wv_sb[:], wv[:])
        nc.sync.dma_start(cos_sb[:, 0:1024], cosT[:, 0:1024])
        nc.sync.dma_start(sin_sb[:, 0:1024], sinT[:, 0:1024])
        nc.scalar.dma_start(tri_sb[:], tri[:])
        nc.scalar.dma_start(idn_sb[:], idn[:])

        def dma_batch_b():
            for cb in range(4):
                nc.sync.dma_start(x_sb[cb][:, :, 1024:2048], xP[cb][:, :, 1024:2048])
            nc.sync.dma_start(cos_sb[:, 1024:2048], cosT[:, 1024:2048])
            nc.sync.dma_start(sin_sb[:, 1024:2048], sinT[:, 1024:2048])
            nc.sync.dma_start(wo_sb[:], wo[:])
            nc.sync.dma_start(wof_sb[:], wof[:])

        scp = ctx.enter_context(tc.tile_pool(name="scps", bufs=2, space="PSUM"))
        pvp = ctx.enter_context(tc.tile_pool(name="pvps", bufs=1, space="PSUM"))
        pop = ctx.enter_context(tc.tile_pool(name="pops", bufs=1, space="PSUM"))
        tmp = ctx.enter_context(tc.tile_pool(name="qktmp", bufs=4))
        swp = ctx.enter_context(tc.tile_pool(name="qkswp", bufs=4))
        exp_pool = ctx.enter_context(tc.tile_pool(name="expool", bufs=4))
        ex16_pool = ctx.enter_context(tc.tile_pool(name="ex16pool", bufs=4))
        atp = ctx.enter_context(tc.tile_pool(name="atp", bufs=2))
        rcp_pool = ctx.enter_context(tc.tile_pool(name="rcppool", bufs=4))
        rbp = ctx.enter_context(tc.tile_pool(name="rbp", bufs=4))
        yp = ctx.enter_context(tc.tile_pool(name="ybuf", bufs=3))

        # ---------------- QKV + RoPE producers -----------------
        def qk_group(kind, t, half):
            if kind == "q":
                wsb, wfsb, rot = wq_sb, wqf_sb, qrot
            else:
                wsb, wfsb, rot = wk_sb, wkf_sb, krot
            s0 = half * 1024
            ps = scp.tile([P, 1024], f32, tag="sc", name="sc")
            for n2 in range(2):
                lo = s0 + n2 * 512
                if lo == 0:
                    # fp16 path: seq cols 0:512 (short causal rows downstream)
                    for k8 in range(8):
                        nc.tensor.matmul(
                            ps[:, 0:512],
                            wfsb[:, k8 * 256 + t * P: k8 * 256 + (t + 1) * P],
                            xf_sb[:, k8, :],
                            start=(k8 == 0), stop=(k8 == 7),
                        )
                else:
                    for cb in range(4):
                        nc.tensor.matmul(
                            ps[:, n2 * 512:(n2 + 1) * 512],
                            wsb[:, :, cb * 256 + t * P: cb * 256 + (t + 1) * P],
                            x_sb[cb][:, :, lo:lo + 512],
                            start=(cb == 0), stop=(cb == 3),
                            perf_mode=DR,
                        )
            span = slice(s0, s0 + 1024)
            base = tmp.tile([P, 1024], fp16, tag="base", name="base")
            if kind == "q":
                nc.vector.tensor_scalar(
                    base[:], ps[:], 1.0 / W_SCALE, qb_sb[t][:, 0:1],
                    mybir.AluOpType.mult, mybir.AluOpType.add,
                )
            else:
                nc.vector.tensor_scalar_mul(base[:], ps[:], 1.0 / W_SCALE)
            sw = swp.tile([P, 1024], fp16, tag="sw", name="sw")
            for blk in range(4):
                sb_, db = 32 * (blk ^ 1), 32 * blk
                nc.sync.dma_start(sw[db:db + 32, :], base[sb_:sb_ + 32, :])
            nc.vector.tensor_mul(base[:], base[:], cos_sb[:, span])
            nc.vector.tensor_mul(sw[:], sw[:], sin_sb[:, span])
            nc.vector.tensor_add(rot[t][:, span], base[:], sw[:])

        def v_group(st):
            ps = scp.tile([P, 1024], f32, tag="sc", name="sc")
            if st < 4:
                # fp16 path feeding chunk 0
                for k8 in range(8):
                    nc.tensor.matmul(
                        ps[:, 0:260],
                        xf_sb[:, k8, st * P:(st + 1) * P],
                        wvf_sb[:, k8 * 260:(k8 + 1) * 260],
                        start=(k8 == 0), stop=(k8 == 7),
                    )
                nc.vector.scalar_tensor_tensor(
                    v16[st][:], ps[:, 0:260], 1.0 / W_SCALE, vb_sb[:],
                    mybir.AluOpType.mult, mybir.AluOpType.add,
                )
            else:
                for sl in range(8):
                    nc.tensor.matmul(
                        ps[:, 0:260],
                        x_sb[sl // 2][:, sl % 2, st * P:(st + 1) * P],
                        wv_sb[:, sl * 260:(sl + 1) * 260],
                        start=(sl == 0), stop=(sl == 7),
                    )
            dst = v_pk[st // 2][:, st % 2, :].rearrange(
                "p (h c) -> p h c", h=4)[:, :, 0:65]
            src = (v16[st][:] if st < 4 else ps[:, 0:260]).rearrange(
                "p (h c) -> p h c", h=4)
            if st < 4:
                nc.vector.tensor_copy(dst, src)
            else:
                vbr = vb_sb[:].rearrange("p (h c) -> p h c", h=4)
                nc.vector.scalar_tensor_tensor(
                    dst, src, 1.0 / W_SCALE, vbr,
                    mybir.AluOpType.mult, mybir.AluOpType.add,
                )

        def qkv_half(half, kinds=("q0", "k0", "q1", "k1"), st0=None):
            sts = iter(range(st0 if st0 is not None else 8 * half, 8 * half + 8))
            for kt_ in kinds:
                qk_group(kt_[0], int(kt_[1]), half)
                v_group(next(sts))
                v_group(next(sts))

        def group_closures(kind, t, half, sts):
            """Closures emitting one half-1 qk group + its two V groups."""
            return [lambda: qk_group(kind, t, half),
                    lambda: v_group(sts[0]),
                    lambda: v_group(sts[1])]

        def drain_one(fifo):
            """Pop hook closures until one unit of PE work has been emitted
            (weight-0 closures carry no PE work and flow through free)."""
            while fifo:
                w, fn = fifo.pop(0)
                fn()
                if w:
                    return

        def drain_until(fifo, label):
            while fifo:
                w, fn = fifo.pop(0)
                fn()
                if fn.__name__ == label or getattr(fn, "label", None) == label:
                    return

        # ---------------- attention (one head-pair of one q-chunk) ------
        def attn_pair(n, pair, at_t, hook=None, tail_hook=None, defer_pv=None):
            """Emit scores/exp/PV/norm for (q-chunk n, head pair); `hook` is
            a FIFO of closures (projections, half-1 QKV group parts, V
            parts) drained one per scores step -- their PE work rides the
            per-step ACT slack instead of creating softmax droughts.
            `tail_hook` fires after the k loop, before the normalization."""
            if hook is None:
                hook = []
            f16 = (n == 0)
            if True:
                nkt = 4 * n + 4
                pv_ps = [pvp.tile([65, 512], f32, tag=f"pv{hh}", name=f"pv{hh}")
                         for hh in range(2)]

                def scores_step(kt, ex_t):
                    diag = (kt // 4 == n)
                    off = P * (kt % 4) if diag else 0
                    sc = scp.tile([P, 1024], f32, tag="sc", name="sc")
                    for hh in range(2):
                        nc.tensor.matmul(
                            sc[:, hh * 512 + off:(hh + 1) * 512],
                            krot[pair][64 * hh:64 * hh + 64, kt * P:(kt + 1) * P],
                            qrot[pair][64 * hh:64 * hh + 64,
                                       n * 512 + off:(n + 1) * 512],
                            start=True, stop=not diag,
                            tile_position=(64 * hh, 0),
                        )
                    if diag:
                        for hh in range(2):
                            nc.tensor.matmul(
                                sc[:, hh * 512 + off:hh * 512 + off + P],
                                idn_sb[:], tri_sb[:],
                                start=False, stop=True,
                            )
                    ex_ap = ex_t[:] if f16 else ex_t[:, kt % 2, :]
                    if diag:
                        exr = ex_ap.rearrange("p (h c) -> p h c", h=2)[:, :, off:]
                        scr = sc[:].rearrange("p (h c) -> p h c", h=2)[:, :, off:]
                        nc.scalar.activation(
                            exr, scr, mybir.ActivationFunctionType.Exp,
                            scale=0.125,
                        )
                        if not f16 and kt % 2 == 1:
                            moff = P * 2 * ((kt % 4) // 2)
                            gap = ex_t[:, 1, :].rearrange(
                                "p (h c) -> p h c", h=2)[:, :, moff:moff + P]
                            nc.gpsimd.memset(gap, 0)
                    else:
                        nc.scalar.activation(
                            ex_ap, sc[:], mybir.ActivationFunctionType.Exp,
                            scale=0.125,
                        )

                if f16:
                    # per-k-tile fp16 PV, 1-tile software pipeline
                    def pv16_step(kt, ex_t):
                        off = P * (kt % 4)
                        for hh in range(2):
                            h = 2 * pair + hh
                            nc.tensor.matmul(
                                pv_ps[hh][:, off:512],
                                v16[kt][:, 65 * h:65 * h + 65],
                                ex_t[:, hh * 512 + off:(hh + 1) * 512],
                                start=(kt == 0), stop=(kt == nkt - 1),
                            )

                    if defer_pv is not None:
                        # scores/exp first; V projection + all PVs after, so
                        # nothing sits between the prefix and the first exp
                        exs = []
                        for kt in range(nkt):
                            ex_t = ex16_pool.tile([P, 1024], fp16,
                                                  tag="ex16", name="ex16")
                            scores_step(kt, ex_t)
                            exs.append((kt, ex_t))
                        for fn in defer_pv:
                            fn()
                        for kt, ex_t in exs:
                            pv16_step(kt, ex_t)
                    else:
                        pend = []
                        for kt in range(nkt):
                            ex_t = ex16_pool.tile([P, 1024], fp16,
                                                  tag="ex16", name="ex16")
                            scores_step(kt, ex_t)
                            if kt >= 1:
                                drain_one(hook)
                            pend.append((kt, ex_t))
                            if len(pend) > 2:
                                pv16_step(*pend.pop(0))
                        for pp_ in pend:
                            pv16_step(*pp_)
                else:
                    def pv_step(sp, ex_t):
                        diag = (sp // 2 == n)
                        moff = P * 2 * (sp % 2) if diag else 0
                        for hh in range(2):
                            h = 2 * pair + hh
                            nc.tensor.matmul(
                                pv_ps[hh][:, moff:512],
                                v_pk[sp][:, :, 80 * h:80 * h + 65],
                                ex_t[:, :, hh * 512 + moff:(hh + 1) * 512],
                                start=(sp == 0), stop=(sp == 2 * n + 1),
                                perf_mode=DR,
                            )

                    pend = []
                    ex_t = None
                    for kt in range(nkt):
                        if kt % 2 == 0:
                            ex_t = exp_pool.tile([P, 2, 1024], fp8, tag="ex", name="ex")
                        scores_step(kt, ex_t)
                        if kt >= 1:
                            drain_one(hook)
                        if kt % 2 == 1:
                            pend.append((kt // 2, ex_t))
                            if len(pend) > 2:
                                pv_step(*pend.pop(0))
                    for pp_ in pend:
                        pv_step(*pp_)

                if tail_hook is not None:
                    tail_hook()
                for hh in range(2):
                    dc = rcp_pool.tile([1, 512], f32, tag="dc", name="dc")
                    nc.vector.tensor_copy(dc[:], pv_ps[hh][64:65, :])
                    rc = rcp_pool.tile([1, 512], f32, tag="rc", name="rc")
                    nc.vector.reciprocal_approx_fast(rc[:], dc[:])
                    rb = rbp.tile([64, 512], f32, tag="rb", name="rb")
                    nc.gpsimd.partition_broadcast(rb[:], rc[0:1, :])
                    nc.vector.tensor_mul(
                        at_t[64 * hh:64 * hh + 64, pair, :],
                        pv_ps[hh][0:64, :],
                        rb[:],
                    )

        def make_proj(n, at_t, last=False):
            f16 = (n == 0)

            def proj_i(i):
                po = pop.tile([P, C], f32, tag="po", name="po")
                for c2 in range(2):
                    if f16:
                        for kk in range(2):
                            nc.tensor.matmul(
                                po[:, c2 * 512:(c2 + 1) * 512],
                                at_t[:, kk, i * P:(i + 1) * P],
                                wof_sb[:, kk, c2 * 512:(c2 + 1) * 512],
                                start=(kk == 0), stop=(kk == 1),
                            )
                    else:
                        nc.tensor.matmul(
                            po[:, c2 * 512:(c2 + 1) * 512],
                            at_t[:, :, i * P:(i + 1) * P],
                            wo_sb[:, :, c2 * 512:(c2 + 1) * 512],
                            start=True, stop=True,
                            perf_mode=DR,
                        )
                yo = yp.tile([P, C], fp16, tag="yo", name="yo")
                if last and i % 2 == 0:
                    nc.scalar.activation(
                        yo[:], po[:], mybir.ActivationFunctionType.Copy,
                        scale=1.0 / OUT_SCALE,
                    )
                else:
                    nc.vector.tensor_scalar_mul(yo[:], po[:], 1.0 / OUT_SCALE)
                r0 = n * 512 + i * P
                nc.sync.dma_start(y[r0:r0 + P, :], yo[:])

            return [lambda i=i: proj_i(i) for i in range(4)]

        def new_at(n):
            return atp.tile([P, 2, 512], fp16 if n == 0 else fp8,
                            tag="at", name="at")

        # ---------------- schedule -----------------------------
        # Minimal inline prefix (q0, k0 + V16) before chunk-0 attention;
        # everything else (q1/k1/half-1 group parts, V8 parts, chunk
        # projections) drains through a global weighted FIFO, one PE
        # closure per scores step, riding the ACT-bound softmax slack.
        # FIFO order keeps every hook-psum ring slot's reader emitted
        # before the slot is reused (WAW safety at hps bufs=2).
        q0mm, q0rope = qk_parts("q", 0, 0, nc.scalar, prefix=True)
        k0mm, k0rope = qk_parts("k", 0, 0, nc.sync, prefix=True)
        q1mm, q1rope = qk_parts("q", 1, 0, nc.scalar)
        k1mm, k1rope = qk_parts("k", 1, 0, nc.sync)
        hq0mm, hq0rope = qk_parts("q", 0, 1, nc.sync)
        hk0mm, hk0rope = qk_parts("k", 0, 1, nc.sync)
        hq1mm, hq1rope = qk_parts("q", 1, 1, nc.sync)
        hk1mm, hk1rope = qk_parts("k", 1, 1, nc.sync)
        v8 = {st: v_parts(st) for st in range(4, 16)}

        q0mm(0); q0mm(1)
        q0rope(0, 512); q0rope(512, 1024)
        k0mm(0); k0mm(1)
        k0rope(0, 512); k0rope(512, 1024)
        dma_batch_b()

        def lab(fn, name):
            fn.label = name
            return fn

        fifo = []
        fifo += [(1, v8[4][0]), (1, v8[4][1]), (1, v8[5][0]), (1, v8[5][1])]
        fifo += [(1, v8[6][0]), (1, v8[6][1]), (1, v8[7][0]), (1, v8[7][1])]
        at0 = new_at(0)

        def q1k1():
            q1mm(0); q1mm(1)
            q1rope(0, 512); q1rope(512, 1024)
            k1mm(0); k1mm(1)
            k1rope(0, 512); k1rope(512, 1024)

        attn_pair(0, 0, at0, hook=fifo,
                  defer_pv=[q1k1] + [lambda st=st: v_group(st) for st in range(4)])
        attn_pair(0, 1, at0, hook=fifo)

        at1 = new_at(1)
        proj0 = make_proj(0, at0)
        fifo += [(1, lambda: hq0mm(0)), (1, lambda: hq0mm(1)),
                 (0, lambda: hq0rope(0, 512)), (0, lambda: hq0rope(512, 1024)),
                 (1, lambda: hk0mm(0)), (1, lambda: hk0mm(1)),
                 (0, lambda: hk0rope(0, 512)),
                 (0, lab(lambda: hk0rope(512, 1024), "hk0done"))]
        fifo += [(1, p) for p in proj0]
        fifo += [(1, v8[8][0])]
        attn_pair(1, 0, at1, hook=fifo)
        attn_pair(1, 1, at1, hook=fifo)
        drain_until(fifo, "hk0done")

        at2 = new_at(2)
        proj1 = make_proj(1, at1)
        fifo += [(1, lambda: hq1mm(0)), (1, lambda: hq1mm(1)),
                 (0, lambda: hq1rope(0, 512)), (0, lambda: hq1rope(512, 1024)),
                 (1, lambda: hk1mm(0)), (1, lambda: hk1mm(1)),
                 (0, lambda: hk1rope(0, 512)),
                 (0, lab(lambda: hk1rope(512, 1024), "hk1done"))]
        fifo += [(1, v8[8][1]), (1, v8[9][0]), (1, v8[9][1]),
                 (1, v8[10][0]), (1, v8[10][1]), (1, v8[11][0]), (1, v8[11][1])]
        attn_pair(2, 0, at2, hook=fifo)
        drain_until(fifo, "hk1done")
        fifo += [(1, p) for p in proj1]
        fifo += [(1, v8[12][0]), (1, v8[12][1]), (1, v8[13][0]), (1, v8[13][1]),
                 (1, v8[14][0]), (1, v8[14][1]), (1, v8[15][0]), (1, v8[15][1])]
        attn_pair(2, 1, at2, hook=fifo)

        at3 = new_at(3)
        proj2 = make_proj(2, at2)
        fifo += [(1, p) for p in proj2]
        attn_pair(3, 0, at3, hook=fifo)
        attn_pair(3, 1, at3, hook=fifo)
        while fifo:
            drain_one(fifo)
        for p in make_proj(3, at3, last=True):
            p()


def _host_inputs(x, w_qkv, q_bias, v_bias, w_out):
    """Build the 8 per-core input maps."""
    import ml_dtypes
    e4m3 = ml_dtypes.float8_e4m3fn
    half = D // 2
    perm64 = np.empty(D, dtype=np.int64)
    perm64[:half] = 2 * np.arange(half)
    perm64[half:] = 2 * np.arange(half) + 1

    dim_t = (TEMP ** (np.arange(half, dtype=np.float32) / half)).astype(np.float32)
    ang = (np.arange(S, dtype=np.float32)[None, :] / dim_t[:, None]).astype(np.float32)
    cos32 = np.cos(ang).astype(np.float32)
    sin32 = np.sin(ang).astype(np.float32)
    cosT = np.tile(cos32, (4, 1)).astype(np.float16)
    sinT = np.concatenate([-sin32, sin32, -sin32, sin32], axis=0).astype(np.float16)

    r = np.arange(P)
    tri = np.where(r[None, :] >= r[:, None], 0.0, -1e9).astype(ml_dtypes.bfloat16)
    idn = np.eye(P, dtype=np.float32).astype(ml_dtypes.bfloat16)

    Wq = w_qkv[:, 0:C]
    Wk = w_qkv[:, C:2 * C]
    Wv = w_qkv[:, 2 * C:3 * C]

    def pack_dr(A):
        # [1024, 256] -> [128, 2, 1024]: out[i, j, cb*256 + m] = A[256cb+i+128j, m]
        return np.ascontiguousarray(
            A.reshape(4, 2, P, 256).transpose(2, 1, 0, 3).reshape(P, 2, 1024)
        )

    def pack16(A):
        # [1024, 256] -> [128, 2048]: out[i, k*256 + m] = A[128k+i, m]
        return np.ascontiguousarray(
            A.reshape(8, P, 256).transpose(1, 0, 2).reshape(P, 2048)
        )

    in_maps = []
    for core in range(N_CORES):
        b, g = core // HPC, core % HPC
        h0 = HPC * g
        cols = np.concatenate([64 * h + perm64 for h in range(h0, h0 + HPC)])
        vcols = np.arange(64 * h0, 64 * h0 + 256)

        xb = np.ascontiguousarray(x[b].T)                     # [C, S]
        xP = np.ascontiguousarray(
            xb.reshape(4, 2, P, S).transpose(0, 2, 1, 3)
        ).astype(e4m3)                                        # [4, 128, 2, S]
        xF = np.ascontiguousarray(
            xb[:, 0:512].reshape(8, P, 512).transpose(1, 0, 2)).astype(np.float16)

        Aq = W_SCALE * Wq[:, cols]
        Ak = W_SCALE * Wk[:, cols]
        wq_pk = pack_dr(Aq).astype(e4m3)
        wk_pk = pack_dr(Ak).astype(e4m3)
        wqf_pk = pack16(Aq).astype(np.float16)
        wkf_pk = pack16(Ak).astype(np.float16)

        wv260 = np.zeros((C, 260), dtype=np.float32)
        vb260 = np.zeros((1, 260), dtype=np.float32)
        wvc = Wv[:, vcols]
        vbc = v_bias[vcols]
        for hh_ in range(4):
            wv260[:, 65 * hh_:65 * hh_ + 64] = wvc[:, 64 * hh_:64 * hh_ + 64]
            vb260[0, 65 * hh_:65 * hh_ + 64] = vbc[64 * hh_:64 * hh_ + 64]
            vb260[0, 65 * hh_ + 64] = 1.0 / DEN_SCALE
        Av = W_SCALE * wv260
        # [1024, 260] -> [128, 2080]: out[i, (2cb+j)*260+m] = Av[256cb+i+128j, m]
        wv_pk = np.ascontiguousarray(
            Av.reshape(4, 2, P, 260).transpose(2, 0, 1, 3).reshape(P, 2080)
        ).astype(e4m3)
        # fp16: out[i, k*260+m] = Av[128k+i, m]
        wvf_pk = np.ascontiguousarray(
            Av.reshape(8, P, 260).transpose(1, 0, 2).reshape(P, 2080)
        ).astype(np.float16)

        Ao = (W_SCALE * w_out[64 * h0:64 * h0 + 256, :]).reshape(2, P, C).transpose(1, 0, 2)
        wo_pk = np.ascontiguousarray(Ao).astype(e4m3)          # [128, 2, 1024]
        wof_pk = np.ascontiguousarray(Ao).astype(np.float16)

        in_maps.append({
            "xP": xP, "xF": xF,
            "wq": wq_pk, "wk": wk_pk, "wqf": wqf_pk, "wkf": wkf_pk,
            "wv": wv_pk, "wvf": wvf_pk, "wo": wo_pk, "wof": wof_pk,
            "qb": np.ascontiguousarray(q_bias[cols].reshape(2, P, 1)),
            "vb": vb260,
            "cosT": cosT, "sinT": sinT, "tri": tri, "idn": idn,
        })
    return in_maps


def kernel(x, w_qkv, q_bias, v_bias, w_out, _trace=False):
    global _NC
    if _NC is None:
        _NC = _build()
    in_maps = _host_inputs(
        np.asarray(x, np.float32), np.asarray(w_qkv, np.float32),
        np.asarray(q_bias, np.float32), np.asarray(v_bias, np.float32),
        np.asarray(w_out, np.float32),
    )
    res = run_bass_kernel_spmd(_NC, in_maps, list(range(N_CORES)), trace=_trace)
    out = np.empty((B, S, C), dtype=np.float32)
    for b in range(B):
        acc = res.results[HPC * b]["y"].astype(np.float32)
        for g in range(1, HPC):
            acc = acc + res.results[HPC * b + g]["y"].astype(np.float32)
        out[b] = acc
    if _trace:
        kernel.last_exec_time_ns = res.exec_time_ns
    return out
